# revision 21
# baseline (speedup 1.0000x reference)
"""TRN2 Bass kernel for GPT-style causal self-attention with RoPE.

Reference (B=2, S=2048, D=1024, H=16, dk=64):
  qkv = hidden @ c_attn_w + c_attn_b; rope(q), rope(k) via position_ids;
  out = softmax(causal(q k^T / 8)) v, merged heads, @ c_proj_w + c_proj_b.

Sharding across 8 NeuronCores: core c = 4*b + g handles batch b and head
group g (4 heads = 256 dims). Each core computes its full S x S attention
and a row-sliced c_proj partial; the host sums the 4 partials per batch.

All matmul operands are bf16 (host converts inputs); PSUM stays f32.
Pipeline per core:
  1. QKV in natural layout (lhsT = host-transposed hidden chunks, DMAs
     issued in parallel from four engine queues); bias via K=1 ones-row
     matmul; rope via 2 DVE multiplies (pair-swap AP, trig broadcast over
     heads) + DVE add; one f32r PE transpose per 128-block; PSUM evictions
     cast to bf16 (Scalar).
  2. Attention blocks (c, hp) in DESCENDING size order, software-pipelined
     at kb granularity: PE alternates scores(i, kb) with PV(i-1, kb) so exp
     (Scalar) is never the serializer. Scores via K=64 matmul pairs (two
     heads in PE quadrants); causal mask folded in as a -240 bias PSUM
     accumulate matmul on the diagonal blocks; exp on Scalar (scale=1/8,
     bf16 out); PV accumulates [v | ones] so row 64 = softmax denominators;
     o_p evicted to SBUF immediately (frees the PSUM slot), then normalize
     via reciprocal_approx_fast + partition_broadcast + DVE multiply into
     per-chunk aT2 tiles (h2=1 via small DMA partition hop).
  3. Projection per 512-col chunk interleaved into the attention stream as
     soon as both head-pairs of that chunk are normalized; c_proj bias added
     by DVE tensor_scalar during PSUM eviction; bf16 output DMA per chunk.
Output per core: outT [1024, 2048] bf16 partial; host sums, transposes.
"""

from contextlib import ExitStack

import numpy as np
import ml_dtypes

import concourse.bacc as bacc
import concourse.tile as tile
import concourse.mybir as mybir
from concourse.bass_utils import run_bass_kernel_spmd

f32 = mybir.dt.float32
f32r = mybir.dt.float32r
bf16 = mybir.dt.bfloat16
AF = mybir.ActivationFunctionType
ALU = mybir.AluOpType

S = 2048
D = 1024
HD = 256           # head dims per core (4 heads x 64)
SB = S // 128      # 16
KC = D // 128      # 8
NCH = S // 512     # 4


def build_attention_nc(num_devices=8):
    nc = bacc.Bacc("TRN2", target_bir_lowering=False, debug=False,
                   num_devices=num_devices)

    hT_d = nc.dram_tensor("hT", [D, S], bf16, kind="ExternalInput")
    wqkv_d = nc.dram_tensor("wqkv", [D, 768], bf16, kind="ExternalInput")
    bqkv_d = nc.dram_tensor("bqkv", [1, 768], bf16, kind="ExternalInput")
    cos_d = nc.dram_tensor("cosb", [S, 64], f32, kind="ExternalInput")
    sin_d = nc.dram_tensor("sinb", [S, 64], f32, kind="ExternalInput")
    wp_d = nc.dram_tensor("wp", [HD, D], bf16, kind="ExternalInput")
    bp_d = nc.dram_tensor("bp", [128, 8], f32, kind="ExternalInput")
    maskb_d = nc.dram_tensor("maskb", [128, 128], bf16, kind="ExternalInput")
    identb_d = nc.dram_tensor("identb", [128, 128], bf16, kind="ExternalInput")
    ident_d = nc.dram_tensor("ident", [128, 128], f32r, kind="ExternalInput")
    onesrow_d = nc.dram_tensor("ones_row", [1, 128], bf16, kind="ExternalInput")
    outT_d = nc.dram_tensor("outT", [D, S], bf16, kind="ExternalOutput")

    with tile.TileContext(nc) as tc, ExitStack() as top:
        const = top.enter_context(tc.tile_pool(name="const", bufs=1))
        ident = const.tile([128, 128], f32r, tag="ident")
        maskb = const.tile([128, 128], bf16, tag="maskb")
        identb = const.tile([128, 128], bf16, tag="identb")
        ones_row = const.tile([1, 128], bf16, tag="ones_row")
        bp_sb = const.tile([128, 8], f32, tag="bp")

        persist = top.enter_context(tc.tile_pool(name="persist", bufs=1))
        qT = [persist.tile([128, S], bf16, tag=f"qT{hp}", name=f"qT{hp}")
              for hp in range(2)]
        kT = [persist.tile([128, S], bf16, tag=f"kT{hp}", name=f"kT{hp}")
              for hp in range(2)]
        v_sb = persist.tile([128, SB, 4, 65], bf16, tag="v")
        wp_sb = persist.tile([128, 2, D], bf16, tag="wp")
        # per-chunk attention outputs (transposed): rows 0:64 = head h2=0,
        # rows 64:128 = h2=1 (DMA hop from aTo)
        aT2 = [[persist.tile([128, 512], bf16, tag=f"aT2_{hp}_{c}",
                             name=f"aT2_{hp}_{c}") for c in range(NCH)]
               for hp in range(2)]
        aTo = [[persist.tile([64, 512], bf16, tag=f"aTo_{hp}_{c}",
                             name=f"aTo_{hp}_{c}") for c in range(NCH)]
               for hp in range(2)]

        # ============ stage 1: QKV + rope + transpose ============
        with ExitStack() as st1:
            hT_pool = st1.enter_context(tc.tile_pool(name="hT", bufs=1))
            w_pool = st1.enter_context(tc.tile_pool(name="w", bufs=1))
            trig_pool = st1.enter_context(tc.tile_pool(name="trig", bufs=4))
            rope_pool = st1.enter_context(tc.tile_pool(name="rope", bufs=1))
            qkv_ps = st1.enter_context(
                tc.tile_pool(name="qkv_ps", bufs=2, space="PSUM"))
            tr_ps = st1.enter_context(
                tc.tile_pool(name="tr_ps", bufs=2, space="PSUM"))

            # -------- input DMAs: one queue, strict priority order, with
            # the critical w/hT0 transfers split so compute starts at the
            # earliest chunk instead of after the full tensor
            w_sb = w_pool.tile([128, KC, 768], bf16, tag="w")
            hT_sb = [hT_pool.tile([128, KC, 512], bf16, tag=f"hT{sg}",
                                  name=f"hT{sg}") for sg in range(NCH)]
            trig = []
            for sg in range(NCH):
                cos_t = trig_pool.tile([128, 4, 64], f32, tag="cos",
                                       name=f"cos{sg}")
                sin_t = trig_pool.tile([128, 4, 64], f32, tag="sin",
                                       name=f"sin{sg}")
                trig.append((cos_t, sin_t))
            bqkv_sb = w_pool.tile([1, 768], bf16, tag="bqkv")
            nc.gpsimd.memset(v_sb[:, :, :, 64], 1.0)

            w_ap = wqkv_d.ap().rearrange("(kc p) f -> p kc f", kc=KC)

            def hT_ap(sg):
                return hT_d.ap()[:, sg * 512:(sg + 1) * 512].rearrange(
                    "(kc p) s -> p kc s", kc=KC)

            def trig_dma(sg):
                cos_t, sin_t = trig[sg]
                nc.sync.dma_start(
                    cos_t[:],
                    cos_d.ap()[sg * 512:(sg + 1) * 512, :].rearrange(
                        "(sbl p) d -> p sbl d", sbl=4))
                nc.sync.dma_start(
                    sin_t[:],
                    sin_d.ap()[sg * 512:(sg + 1) * 512, :].rearrange(
                        "(sbl p) d -> p sbl d", sbl=4))

            nc.sync.dma_start(w_sb[:, 0:4, :], w_ap[:, 0:4, :])
            nc.sync.dma_start(hT_sb[0][:, 0:4, :], hT_ap(0)[:, 0:4, :])
            trig_dma(0)
            nc.sync.dma_start(w_sb[:, 4:8, :], w_ap[:, 4:8, :])
            nc.sync.dma_start(hT_sb[0][:, 4:8, :], hT_ap(0)[:, 4:8, :])
            nc.sync.dma_start(bqkv_sb[:], bqkv_d.ap())
            nc.sync.dma_start(ones_row[:], onesrow_d.ap())
            nc.sync.dma_start(hT_sb[1][:], hT_ap(1))
            nc.sync.dma_start(ident[:], ident_d.ap())
            trig_dma(1)
            nc.sync.dma_start(hT_sb[2][:], hT_ap(2))
            trig_dma(2)
            nc.sync.dma_start(hT_sb[3][:], hT_ap(3))
            trig_dma(3)
            nc.sync.dma_start(maskb[:], maskb_d.ap())
            nc.sync.dma_start(identb[:], identb_d.ap())
            nc.sync.dma_start(
                wp_sb[:], wp_d.ap().rearrange("(kc2 p) f -> p kc2 f", kc2=2))
            nc.sync.dma_start(bp_sb[:], bp_d.ap())

            for sg in range(NCH):
                cos_t, sin_t = trig[sg]
                rope_tiles = {}
                for sbl in range(4):
                    sb = sg * 4 + sbl
                    qkv_p = qkv_ps.tile([128, 768], f32, tag="qkv_p")
                    for kc in range(KC):
                        lhsT = hT_sb[sg][:, kc, sbl * 128:(sbl + 1) * 128]
                        nc.tensor.matmul(qkv_p[:, 0:512], lhsT,
                                         w_sb[:, kc, 0:512],
                                         start=(kc == 0), stop=False)
                        nc.tensor.matmul(qkv_p[:, 512:768], lhsT,
                                         w_sb[:, kc, 512:768],
                                         start=(kc == 0), stop=False)
                    nc.tensor.matmul(qkv_p[:, 0:512], ones_row[:],
                                     bqkv_sb[:, 0:512], start=False, stop=True)
                    nc.tensor.matmul(qkv_p[:, 512:768], ones_row[:],
                                     bqkv_sb[:, 512:768], start=False,
                                     stop=True)

                    cos_b = cos_t[:, sbl:sbl + 1, :].broadcast_to((128, 4, 64))
                    sin_b = sin_t[:, sbl:sbl + 1, :].rearrange(
                        "p o (t d) -> p o t d", t=2).broadcast_to(
                        (128, 4, 2, 32))
                    for qk in range(2):
                        base = qk * HD
                        pin = qkv_p[:, base:base + HD]
                        pin_h = pin.rearrange("p (h d) -> p h d", h=4)
                        pin_sw = pin.rearrange("p (h t d) -> p h t d",
                                               h=4, t=2)[:, :, ::-1, :]
                        t1 = rope_pool.tile([128, HD], f32r,
                                            tag=f"t1_{qk}_{sbl}",
                                            name=f"t1_{qk}_{sbl}")
                        t2 = rope_pool.tile([128, HD], f32r,
                                            tag=f"t2_{qk}_{sbl}",
                                            name=f"t2_{qk}_{sbl}")
                        tt = rope_pool.tile([128, HD], f32r,
                                            tag=f"t_{qk}_{sbl}",
                                            name=f"t_{qk}_{sbl}")
                        nc.vector.tensor_tensor(
                            t1[:].rearrange("p (h d) -> p h d", h=4),
                            pin_h, cos_b, op=ALU.mult)
                        nc.vector.tensor_tensor(
                            t2[:].rearrange("p (h t d) -> p h t d", h=4, t=2),
                            pin_sw, sin_b, op=ALU.mult)
                        nc.gpsimd.tensor_tensor(tt[:], t1[:], t2[:],
                                                op=ALU.add)
                        rope_tiles[(qk, sbl)] = tt

                    nc.scalar.copy(
                        v_sb[:, sb, :, 0:64],
                        qkv_p[:, 512:768].rearrange("p (h d) -> p h d", h=4))

                for qk in range(2):
                    dest = qT if qk == 0 else kT
                    for hp in range(2):
                        tp = tr_ps.tile([128, 512], f32, tag="tr_p")
                        for sbl in range(4):
                            tt = rope_tiles[(qk, sbl)]
                            nc.tensor.matmul(
                                tp[:, sbl * 128:(sbl + 1) * 128].bitcast(f32r),
                                tt[:, hp * 128:(hp + 1) * 128], ident[:],
                                is_transpose=True, start=True, stop=True)
                        nc.scalar.copy(
                            dest[hp][:, sg * 512:(sg + 1) * 512], tp[:])

        # ============ stages 2+3 interleaved ============
        with ExitStack() as st2:
            pt_pool = st2.enter_context(tc.tile_pool(name="pt", bufs=33))
            nrm_pool = st2.enter_context(tc.tile_pool(name="nrm", bufs=4))
            po_pool = st2.enter_context(tc.tile_pool(name="po", bufs=2))
            st_ps = st2.enter_context(
                tc.tile_pool(name="st_ps", bufs=2, space="PSUM"))
            out_ps = st2.enter_context(
                tc.tile_pool(name="out_ps", bufs=2, space="PSUM"))
            pj_ps = st2.enter_context(
                tc.tile_pool(name="pj_ps", bufs=2, space="PSUM"))

            def emit_score(c, hp, kb, pts):
                q0 = max(512 * c, 128 * kb)
                off = q0 - 512 * c
                diag = 128 * kb >= 512 * c
                st_p = st_ps.tile([128, 2, 512], f32, tag="st_p")
                for h2 in range(2):
                    nc.tensor.matmul(
                        st_p[:, h2, off:512],
                        kT[hp][h2 * 64:(h2 + 1) * 64,
                               kb * 128:(kb + 1) * 128],
                        qT[hp][h2 * 64:(h2 + 1) * 64,
                               q0:512 * (c + 1)],
                        start=True, stop=not diag,
                        tile_position=(h2 * 64, 0))
                    if diag:
                        # causal mask: add -240 above the diagonal, pre-exp
                        nc.tensor.matmul(
                            st_p[:, h2, off:off + 128],
                            identb[:], maskb[:],
                            start=False, stop=True)
                pt = pt_pool.tile([128, 2, 512], bf16, tag="pt")
                nc.scalar.activation(pt[:, :, off:512],
                                     st_p[:, :, off:512],
                                     AF.Exp, scale=0.125)
                pts.append((kb, off, pt))

            def emit_pv(hp, kb, off, pt, o_ps, nkb):
                for h2 in range(2):
                    nc.tensor.matmul(
                        o_ps[h2][0:65, off:512],
                        v_sb[:, kb, 2 * hp + h2, :],
                        pt[:, h2, off:512],
                        start=(kb == 0), stop=(kb == nkb - 1))

            def emit_norm(c, hp, o_ps):
                # both h2 denominators processed in single wide ops
                oes = {}
                den0 = nrm_pool.tile([1, 2, 512], f32, tag="den0")
                rcp0 = nrm_pool.tile([1, 2, 512], f32, tag="rcp0")
                bc = nrm_pool.tile([64, 2, 512], f32, tag="bc")
                for h2 in (1, 0):
                    oe = nrm_pool.tile([65, 512], f32, tag="oe")
                    # free the PSUM slot fast, then normalize from SBUF
                    nc.vector.tensor_copy(oe[0:65, :], o_ps[h2][0:65, :])
                    oes[h2] = oe
                for h2 in (1, 0):
                    # custom-DVE recip and partition_broadcast need
                    # partition-0 operands; DMA does the cross-partition hop
                    nc.gpsimd.dma_start(den0[:, h2, :], oes[h2][64:65, :])
                nc.vector.reciprocal_approx_fast(rcp0[:], den0[:])
                nc.gpsimd.partition_broadcast(bc[:], rcp0[:])
                for h2 in (1, 0):
                    out_ap = (aT2[hp][c][0:64, :] if h2 == 0
                              else aTo[hp][c][0:64, :])
                    nc.vector.tensor_tensor(out_ap, oes[h2][0:64, :],
                                            bc[:, h2, :], op=ALU.mult)
                    if h2 == 1:
                        nc.gpsimd.dma_start(aT2[hp][c][64:128, :],
                                            aTo[hp][c][0:64, :])

            def emit_proj(c):
                po = po_pool.tile([128, 8, 512], bf16, tag="po")
                for dd in range(8):
                    pp = pj_ps.tile([128, 512], f32, tag="pp")
                    for kc2 in range(2):
                        nc.tensor.matmul(
                            pp[:],
                            wp_sb[:, kc2, dd * 128:(dd + 1) * 128],
                            aT2[kc2][c][:],
                            start=(kc2 == 0), stop=(kc2 == 1))
                    nc.vector.tensor_scalar_add(po[:, dd, :], pp[:],
                                                bp_sb[:, dd:dd + 1])
                    eng = nc.sync if dd % 2 == 0 else nc.scalar
                    eng.dma_start(
                        outT_d.ap()[dd * 128:(dd + 1) * 128,
                                    c * 512:(c + 1) * 512],
                        po[:, dd, :])

            # ascending block size: exp demand ramps up in step with the
            # PV backlog, so the st-buffer throttle never leaves PE idle;
            # scores(i) interleaved with PV(i-1) at kb granularity so
            # Scalar(exp) never stalls PE; ready proj chunks are dropped
            # into the middle of the kb stream to fill exp bubbles
            blocks = [(c, hp) for c in (0, 1, 2, 3) for hp in (0, 1)]
            prev = None
            proj_q = []
            for i in range(len(blocks) + 1):
                blk = blocks[i] if i < len(blocks) else None
                nkb_i = 4 * blk[0] + 4 if blk else 0
                nkb_p = len(prev[2]) if prev else 0
                o_ps = None
                if prev:
                    o_ps = [out_ps.tile([128, 512], f32, tag="o_p",
                                        name=f"o_p{h2}") for h2 in range(2)]
                pts = []
                proj_c = proj_q.pop(0) if proj_q else None
                for kb in range(max(nkb_i, nkb_p)):
                    if kb < nkb_i:
                        emit_score(blk[0], blk[1], kb, pts)
                    if kb < nkb_p:
                        pkb, poff, ppt = prev[2][kb]
                        emit_pv(prev[1], pkb, poff, ppt, o_ps, nkb_p)
                        if kb == nkb_p - 1:
                            # norm chain queued right behind the last PV so
                            # later GpSimd/DVE work can't head-of-line it
                            emit_norm(prev[0], prev[1], o_ps)
                    if kb == 5 and proj_c is not None:
                        emit_proj(proj_c)
                        proj_c = None
                if proj_c is not None:
                    emit_proj(proj_c)
                if prev:
                    if prev[1] == 1:
                        proj_q.append(prev[0])
                prev = (blk[0], blk[1], pts) if blk else None
            while proj_q:
                emit_proj(proj_q.pop(0))

    nc.finalize()
    return nc


def make_core_inputs(inputs, core):
    """Host-side shard prep for one core."""
    b, g = core // 4, core % 4
    hidden = np.asarray(inputs["hidden_states"], dtype=np.float32)
    pos = np.asarray(inputs["position_ids"])
    caw = np.asarray(inputs["c_attn_w"], dtype=np.float32)
    cab = np.asarray(inputs["c_attn_b"], dtype=np.float32)
    cpw = np.asarray(inputs["c_proj_w"], dtype=np.float32)
    cpb = np.asarray(inputs["c_proj_b"], dtype=np.float32)
    bft = ml_dtypes.bfloat16

    cs = slice(g * HD, (g + 1) * HD)
    wqkv = np.concatenate(
        [caw[:, cs], caw[:, D + g * HD:D + (g + 1) * HD],
         caw[:, 2 * D + g * HD:2 * D + (g + 1) * HD]], axis=1)
    bqkv = np.concatenate(
        [cab[cs], cab[D + g * HD:D + (g + 1) * HD],
         cab[2 * D + g * HD:2 * D + (g + 1) * HD]])[None, :]

    inv_freq = (1.0 / (10000.0 **
                       (np.arange(0, 64, 2, dtype=np.float64) / 64.0)))
    freqs = pos[b].astype(np.float64)[:, None] * inv_freq[None, :]
    emb = np.concatenate([freqs, freqs], axis=1)
    cos = np.cos(emb).astype(np.float32)
    sins = np.sin(emb).astype(np.float32)
    sins[:, :32] *= -1.0

    bp = (cpb if g == 0 else np.zeros_like(cpb)).reshape(8, 128).T.copy()

    r = np.arange(128)
    maskb = np.where(r[None, :] >= r[:, None], 0.0, -240.0)

    return {
        "hT": np.ascontiguousarray(hidden[b].T).astype(bft),
        "wqkv": np.ascontiguousarray(wqkv).astype(bft),
        "bqkv": np.ascontiguousarray(bqkv).astype(bft),
        "cosb": cos,
        "sinb": sins,
        "wp": np.ascontiguousarray(cpw[cs, :]).astype(bft),
        "bp": np.ascontiguousarray(bp.astype(np.float32)),
        "maskb": maskb.astype(bft),
        "identb": np.eye(128, dtype=np.float32).astype(bft),
        "ident": np.eye(128, dtype=np.float32),
        "ones_row": np.ones((1, 128), bft),
    }


_NC_CACHE = {}


def run(inputs, trace=False, **spmd_kwargs):
    """Shard, execute on 8 cores, unshard. Returns (output, BassKernelResults)."""
    if "nc" not in _NC_CACHE:
        _NC_CACHE["nc"] = build_attention_nc(num_devices=8)
    nc = _NC_CACHE["nc"]
    in_maps = [make_core_inputs(inputs, c) for c in range(8)]
    res = run_bass_kernel_spmd(nc, in_maps, core_ids=list(range(8)),
                               trace=trace, **spmd_kwargs)
    outs = []
    for b in range(2):
        acc = np.zeros((D, S), np.float64)
        for g in range(4):
            acc += res.results[b * 4 + g]["outT"].astype(np.float64)
        outs.append(acc.T.astype(np.float32))
    return np.stack(outs, axis=0), res


def kernel(**inputs) -> np.ndarray:
    out, _ = run(inputs, trace=False)
    return out


# revision 22
# speedup vs baseline: 1.0213x; 1.0213x over previous
"""TRN2 Bass kernel for GPT-style causal self-attention with RoPE.

Reference (B=2, S=2048, D=1024, H=16, dk=64):
  qkv = hidden @ c_attn_w + c_attn_b; rope(q), rope(k) via position_ids;
  out = softmax(causal(q k^T / 8)) v, merged heads, @ c_proj_w + c_proj_b.

Sharding across 8 NeuronCores: core c = 4*b + g handles batch b and head
group g (4 heads = 256 dims). Each core computes its full S x S attention
and a row-sliced c_proj partial; the host sums the 4 partials per batch.

All matmul operands are bf16 (host converts inputs); PSUM stays f32.
Pipeline per core:
  1. QKV in natural layout (lhsT = host-transposed hidden chunks, DMAs
     issued in parallel from four engine queues); bias via K=1 ones-row
     matmul; rope via 2 DVE multiplies (pair-swap AP, trig broadcast over
     heads) + DVE add; one f32r PE transpose per 128-block; PSUM evictions
     cast to bf16 (Scalar).
  2. Attention blocks (c, hp) in DESCENDING size order, software-pipelined
     at kb granularity: PE alternates scores(i, kb) with PV(i-1, kb) so exp
     (Scalar) is never the serializer. Scores via K=64 matmul pairs (two
     heads in PE quadrants); causal mask folded in as a -240 bias PSUM
     accumulate matmul on the diagonal blocks; exp on Scalar (scale=1/8,
     bf16 out); PV accumulates [v | ones] so row 64 = softmax denominators;
     o_p evicted to SBUF immediately (frees the PSUM slot), then normalize
     via reciprocal_approx_fast + partition_broadcast + DVE multiply into
     per-chunk aT2 tiles (h2=1 via small DMA partition hop).
  3. Projection per 512-col chunk interleaved into the attention stream as
     soon as both head-pairs of that chunk are normalized; c_proj bias added
     by DVE tensor_scalar during PSUM eviction; bf16 output DMA per chunk.
Output per core: outT [1024, 2048] bf16 partial; host sums, transposes.
"""

from contextlib import ExitStack

import numpy as np
import ml_dtypes

import concourse.bacc as bacc
import concourse.tile as tile
import concourse.mybir as mybir
from concourse.bass_utils import run_bass_kernel_spmd

f32 = mybir.dt.float32
f32r = mybir.dt.float32r
bf16 = mybir.dt.bfloat16
AF = mybir.ActivationFunctionType
ALU = mybir.AluOpType

S = 2048
D = 1024
HD = 256           # head dims per core (4 heads x 64)
SB = S // 128      # 16
KC = D // 128      # 8
NCH = S // 512     # 4


def build_attention_nc(num_devices=8):
    nc = bacc.Bacc("TRN2", target_bir_lowering=False, debug=False,
                   num_devices=num_devices)

    hT_d = nc.dram_tensor("hT", [D, S], bf16, kind="ExternalInput")
    wqkv_d = nc.dram_tensor("wqkv", [D, 768], bf16, kind="ExternalInput")
    bqkv_d = nc.dram_tensor("bqkv", [1, 768], bf16, kind="ExternalInput")
    cos_d = nc.dram_tensor("cosb", [S, 64], f32, kind="ExternalInput")
    sin_d = nc.dram_tensor("sinb", [S, 64], f32, kind="ExternalInput")
    wp_d = nc.dram_tensor("wp", [HD, D], bf16, kind="ExternalInput")
    bp_d = nc.dram_tensor("bp", [128, 8], f32, kind="ExternalInput")
    maskb_d = nc.dram_tensor("maskb", [128, 128], bf16, kind="ExternalInput")
    identb_d = nc.dram_tensor("identb", [128, 128], bf16, kind="ExternalInput")
    ident_d = nc.dram_tensor("ident", [128, 128], f32r, kind="ExternalInput")
    onesrow_d = nc.dram_tensor("ones_row", [1, 128], bf16, kind="ExternalInput")
    outT_d = nc.dram_tensor("outT", [D, S], bf16, kind="ExternalOutput")

    with tile.TileContext(nc) as tc, ExitStack() as top:
        const = top.enter_context(tc.tile_pool(name="const", bufs=1))
        ident = const.tile([128, 128], f32r, tag="ident")
        maskb = const.tile([128, 128], bf16, tag="maskb")
        identb = const.tile([128, 128], bf16, tag="identb")
        ones_row = const.tile([1, 128], bf16, tag="ones_row")
        bp_sb = const.tile([128, 8], f32, tag="bp")

        persist = top.enter_context(tc.tile_pool(name="persist", bufs=1))
        qT = [persist.tile([128, S], bf16, tag=f"qT{hp}", name=f"qT{hp}")
              for hp in range(2)]
        kT = [persist.tile([128, S], bf16, tag=f"kT{hp}", name=f"kT{hp}")
              for hp in range(2)]
        v_sb = persist.tile([128, SB, 4, 65], bf16, tag="v")
        wp_sb = persist.tile([128, 2, D], bf16, tag="wp")
        # per-chunk attention outputs (transposed): rows 0:64 = head h2=0,
        # rows 64:128 = h2=1 (DMA hop from aTo)
        aT2 = [[persist.tile([128, 512], bf16, tag=f"aT2_{hp}_{c}",
                             name=f"aT2_{hp}_{c}") for c in range(NCH)]
               for hp in range(2)]
        aTo = [[persist.tile([64, 512], bf16, tag=f"aTo_{hp}_{c}",
                             name=f"aTo_{hp}_{c}") for c in range(NCH)]
               for hp in range(2)]

        # ============ stage 1: QKV + rope + transpose ============
        with ExitStack() as st1:
            hT_pool = st1.enter_context(tc.tile_pool(name="hT", bufs=1))
            w_pool = st1.enter_context(tc.tile_pool(name="w", bufs=1))
            trig_pool = st1.enter_context(tc.tile_pool(name="trig", bufs=4))
            rope_pool = st1.enter_context(tc.tile_pool(name="rope", bufs=1))
            qkv_ps = st1.enter_context(
                tc.tile_pool(name="qkv_ps", bufs=2, space="PSUM"))
            tr_ps = st1.enter_context(
                tc.tile_pool(name="tr_ps", bufs=2, space="PSUM"))

            # -------- input DMAs: one queue, strict priority order, with
            # the critical w/hT0 transfers split so compute starts at the
            # earliest chunk instead of after the full tensor
            w_sb = w_pool.tile([128, KC, 768], bf16, tag="w")
            hT_sb = [hT_pool.tile([128, KC, 512], bf16, tag=f"hT{sg}",
                                  name=f"hT{sg}") for sg in range(NCH)]
            trig = []
            for sg in range(NCH):
                cos_t = trig_pool.tile([128, 4, 64], f32, tag="cos",
                                       name=f"cos{sg}")
                sin_t = trig_pool.tile([128, 4, 64], f32, tag="sin",
                                       name=f"sin{sg}")
                trig.append((cos_t, sin_t))
            bqkv_sb = w_pool.tile([1, 768], bf16, tag="bqkv")
            nc.gpsimd.memset(v_sb[:, :, :, 64], 1.0)

            w_ap = wqkv_d.ap().rearrange("(kc p) f -> p kc f", kc=KC)

            def hT_ap(sg):
                return hT_d.ap()[:, sg * 512:(sg + 1) * 512].rearrange(
                    "(kc p) s -> p kc s", kc=KC)

            def trig_dma(sg):
                cos_t, sin_t = trig[sg]
                nc.sync.dma_start(
                    cos_t[:],
                    cos_d.ap()[sg * 512:(sg + 1) * 512, :].rearrange(
                        "(sbl p) d -> p sbl d", sbl=4))
                nc.sync.dma_start(
                    sin_t[:],
                    sin_d.ap()[sg * 512:(sg + 1) * 512, :].rearrange(
                        "(sbl p) d -> p sbl d", sbl=4))

            nc.sync.dma_start(w_sb[:, 0:4, :], w_ap[:, 0:4, :])
            nc.sync.dma_start(hT_sb[0][:, 0:4, :], hT_ap(0)[:, 0:4, :])
            trig_dma(0)
            nc.sync.dma_start(w_sb[:, 4:8, :], w_ap[:, 4:8, :])
            nc.sync.dma_start(hT_sb[0][:, 4:8, :], hT_ap(0)[:, 4:8, :])
            nc.sync.dma_start(bqkv_sb[:], bqkv_d.ap())
            nc.sync.dma_start(ones_row[:], onesrow_d.ap())
            nc.sync.dma_start(hT_sb[1][:], hT_ap(1))
            nc.sync.dma_start(ident[:], ident_d.ap())
            trig_dma(1)
            nc.sync.dma_start(hT_sb[2][:], hT_ap(2))
            trig_dma(2)
            nc.sync.dma_start(hT_sb[3][:], hT_ap(3))
            trig_dma(3)
            nc.sync.dma_start(maskb[:], maskb_d.ap())
            nc.sync.dma_start(identb[:], identb_d.ap())
            nc.sync.dma_start(
                wp_sb[:], wp_d.ap().rearrange("(kc2 p) f -> p kc2 f", kc2=2))
            nc.sync.dma_start(bp_sb[:], bp_d.ap())

            for sg in range(NCH):
                cos_t, sin_t = trig[sg]
                rope_tiles = {}
                for sbl in range(4):
                    sb = sg * 4 + sbl
                    qkv_p = qkv_ps.tile([128, 768], f32, tag="qkv_p")
                    for kc in range(KC):
                        lhsT = hT_sb[sg][:, kc, sbl * 128:(sbl + 1) * 128]
                        nc.tensor.matmul(qkv_p[:, 0:512], lhsT,
                                         w_sb[:, kc, 0:512],
                                         start=(kc == 0), stop=False)
                        nc.tensor.matmul(qkv_p[:, 512:768], lhsT,
                                         w_sb[:, kc, 512:768],
                                         start=(kc == 0), stop=False)
                    nc.tensor.matmul(qkv_p[:, 0:512], ones_row[:],
                                     bqkv_sb[:, 0:512], start=False, stop=True)
                    nc.tensor.matmul(qkv_p[:, 512:768], ones_row[:],
                                     bqkv_sb[:, 512:768], start=False,
                                     stop=True)

                    cos_b = cos_t[:, sbl:sbl + 1, :].broadcast_to((128, 4, 64))
                    sin_b = sin_t[:, sbl:sbl + 1, :].rearrange(
                        "p o (t d) -> p o t d", t=2).broadcast_to(
                        (128, 4, 2, 32))
                    for qk in range(2):
                        base = qk * HD
                        pin = qkv_p[:, base:base + HD]
                        pin_h = pin.rearrange("p (h d) -> p h d", h=4)
                        pin_sw = pin.rearrange("p (h t d) -> p h t d",
                                               h=4, t=2)[:, :, ::-1, :]
                        t1 = rope_pool.tile([128, HD], f32r,
                                            tag=f"t1_{qk}_{sbl}",
                                            name=f"t1_{qk}_{sbl}")
                        t2 = rope_pool.tile([128, HD], f32r,
                                            tag=f"t2_{qk}_{sbl}",
                                            name=f"t2_{qk}_{sbl}")
                        tt = rope_pool.tile([128, HD], f32r,
                                            tag=f"t_{qk}_{sbl}",
                                            name=f"t_{qk}_{sbl}")
                        nc.vector.tensor_tensor(
                            t1[:].rearrange("p (h d) -> p h d", h=4),
                            pin_h, cos_b, op=ALU.mult)
                        nc.vector.tensor_tensor(
                            t2[:].rearrange("p (h t d) -> p h t d", h=4, t=2),
                            pin_sw, sin_b, op=ALU.mult)
                        nc.gpsimd.tensor_tensor(tt[:], t1[:], t2[:],
                                                op=ALU.add)
                        rope_tiles[(qk, sbl)] = tt

                    nc.scalar.copy(
                        v_sb[:, sb, :, 0:64],
                        qkv_p[:, 512:768].rearrange("p (h d) -> p h d", h=4))

                for qk in range(2):
                    dest = qT if qk == 0 else kT
                    for hp in range(2):
                        tp = tr_ps.tile([128, 512], f32, tag="tr_p")
                        for sbl in range(4):
                            tt = rope_tiles[(qk, sbl)]
                            nc.tensor.matmul(
                                tp[:, sbl * 128:(sbl + 1) * 128].bitcast(f32r),
                                tt[:, hp * 128:(hp + 1) * 128], ident[:],
                                is_transpose=True, start=True, stop=True)
                        nc.scalar.copy(
                            dest[hp][:, sg * 512:(sg + 1) * 512], tp[:])

        # ============ stages 2+3 interleaved ============
        with ExitStack() as st2:
            pt_pool = st2.enter_context(tc.tile_pool(name="pt", bufs=33))
            nrm_pool = st2.enter_context(tc.tile_pool(name="nrm", bufs=4))
            po_pool = st2.enter_context(tc.tile_pool(name="po", bufs=2))
            st_ps = st2.enter_context(
                tc.tile_pool(name="st_ps", bufs=2, space="PSUM"))
            out_ps = st2.enter_context(
                tc.tile_pool(name="out_ps", bufs=2, space="PSUM"))
            pj_ps = st2.enter_context(
                tc.tile_pool(name="pj_ps", bufs=2, space="PSUM"))

            def emit_score(c, hp, kb, pts):
                q0 = max(512 * c, 128 * kb)
                off = q0 - 512 * c
                diag = 128 * kb >= 512 * c
                st_p = st_ps.tile([128, 2, 512], f32, tag="st_p")
                for h2 in range(2):
                    nc.tensor.matmul(
                        st_p[:, h2, off:512],
                        kT[hp][h2 * 64:(h2 + 1) * 64,
                               kb * 128:(kb + 1) * 128],
                        qT[hp][h2 * 64:(h2 + 1) * 64,
                               q0:512 * (c + 1)],
                        start=True, stop=not diag,
                        tile_position=(h2 * 64, 0))
                    if diag:
                        # causal mask: add -240 above the diagonal, pre-exp
                        nc.tensor.matmul(
                            st_p[:, h2, off:off + 128],
                            identb[:], maskb[:],
                            start=False, stop=True)
                pt = pt_pool.tile([128, 2, 512], bf16, tag="pt")
                nc.scalar.activation(pt[:, :, off:512],
                                     st_p[:, :, off:512],
                                     AF.Exp, scale=0.125)
                pts.append((kb, off, pt))

            def emit_pv(hp, kb, off, pt, o_ps, nkb):
                for h2 in range(2):
                    nc.tensor.matmul(
                        o_ps[h2][0:65, off:512],
                        v_sb[:, kb, 2 * hp + h2, :],
                        pt[:, h2, off:512],
                        start=(kb == 0), stop=(kb == nkb - 1))

            def emit_norm(c, hp, o_ps):
                # both h2 denominators processed in single wide ops
                oes = {}
                den0 = nrm_pool.tile([1, 2, 512], f32, tag="den0")
                rcp0 = nrm_pool.tile([1, 2, 512], f32, tag="rcp0")
                bc = nrm_pool.tile([64, 2, 512], f32, tag="bc")
                for h2 in (1, 0):
                    oe = nrm_pool.tile([65, 512], f32, tag="oe")
                    # free the PSUM slot fast, then normalize from SBUF
                    nc.vector.tensor_copy(oe[0:65, :], o_ps[h2][0:65, :])
                    oes[h2] = oe
                for h2 in (1, 0):
                    # custom-DVE recip and partition_broadcast need
                    # partition-0 operands; DMA does the cross-partition hop
                    nc.gpsimd.dma_start(den0[:, h2, :], oes[h2][64:65, :])
                nc.vector.reciprocal_approx_fast(rcp0[:], den0[:])
                nc.gpsimd.partition_broadcast(bc[:], rcp0[:])
                for h2 in (1, 0):
                    out_ap = (aT2[hp][c][0:64, :] if h2 == 0
                              else aTo[hp][c][0:64, :])
                    nc.vector.tensor_tensor(out_ap, oes[h2][0:64, :],
                                            bc[:, h2, :], op=ALU.mult)
                    if h2 == 1:
                        nc.gpsimd.dma_start(aT2[hp][c][64:128, :],
                                            aTo[hp][c][0:64, :])

            def emit_proj(c):
                po = po_pool.tile([128, 8, 512], bf16, tag="po")
                for dd in range(8):
                    pp = pj_ps.tile([128, 512], f32, tag="pp")
                    for kc2 in range(2):
                        nc.tensor.matmul(
                            pp[:],
                            wp_sb[:, kc2, dd * 128:(dd + 1) * 128],
                            aT2[kc2][c][:],
                            start=(kc2 == 0), stop=(kc2 == 1))
                    nc.vector.tensor_scalar_add(po[:, dd, :], pp[:],
                                                bp_sb[:, dd:dd + 1])
                    nc.sync.dma_start(
                        outT_d.ap()[dd * 128:(dd + 1) * 128,
                                    c * 512:(c + 1) * 512],
                        po[:, dd, :])

            # ascending block size: exp demand ramps up in step with the
            # PV backlog, so the st-buffer throttle never leaves PE idle;
            # scores(i) interleaved with PV(i-1) at kb granularity so
            # Scalar(exp) never stalls PE; ready proj chunks are dropped
            # into the middle of the kb stream to fill exp bubbles
            blocks = [(c, hp) for c in (0, 1, 2, 3) for hp in (0, 1)]
            prev = None
            proj_q = []
            for i in range(len(blocks) + 1):
                blk = blocks[i] if i < len(blocks) else None
                nkb_i = 4 * blk[0] + 4 if blk else 0
                nkb_p = len(prev[2]) if prev else 0
                o_ps = None
                if prev:
                    o_ps = [out_ps.tile([128, 512], f32, tag="o_p",
                                        name=f"o_p{h2}") for h2 in range(2)]
                pts = []
                proj_c = proj_q.pop(0) if proj_q else None
                for kb in range(max(nkb_i, nkb_p)):
                    if kb < nkb_i:
                        emit_score(blk[0], blk[1], kb, pts)
                    if kb < nkb_p:
                        pkb, poff, ppt = prev[2][kb]
                        emit_pv(prev[1], pkb, poff, ppt, o_ps, nkb_p)
                        if kb == nkb_p - 1:
                            # norm chain queued right behind the last PV so
                            # later GpSimd/DVE work can't head-of-line it
                            emit_norm(prev[0], prev[1], o_ps)
                    if kb == 5 and proj_c is not None:
                        emit_proj(proj_c)
                        proj_c = None
                if proj_c is not None:
                    emit_proj(proj_c)
                if prev:
                    if prev[1] == 1:
                        proj_q.append(prev[0])
                prev = (blk[0], blk[1], pts) if blk else None
            while proj_q:
                emit_proj(proj_q.pop(0))

    nc.finalize()
    return nc


def make_core_inputs(inputs, core):
    """Host-side shard prep for one core."""
    b, g = core // 4, core % 4
    hidden = np.asarray(inputs["hidden_states"], dtype=np.float32)
    pos = np.asarray(inputs["position_ids"])
    caw = np.asarray(inputs["c_attn_w"], dtype=np.float32)
    cab = np.asarray(inputs["c_attn_b"], dtype=np.float32)
    cpw = np.asarray(inputs["c_proj_w"], dtype=np.float32)
    cpb = np.asarray(inputs["c_proj_b"], dtype=np.float32)
    bft = ml_dtypes.bfloat16

    cs = slice(g * HD, (g + 1) * HD)
    wqkv = np.concatenate(
        [caw[:, cs], caw[:, D + g * HD:D + (g + 1) * HD],
         caw[:, 2 * D + g * HD:2 * D + (g + 1) * HD]], axis=1)
    bqkv = np.concatenate(
        [cab[cs], cab[D + g * HD:D + (g + 1) * HD],
         cab[2 * D + g * HD:2 * D + (g + 1) * HD]])[None, :]

    inv_freq = (1.0 / (10000.0 **
                       (np.arange(0, 64, 2, dtype=np.float64) / 64.0)))
    freqs = pos[b].astype(np.float64)[:, None] * inv_freq[None, :]
    emb = np.concatenate([freqs, freqs], axis=1)
    cos = np.cos(emb).astype(np.float32)
    sins = np.sin(emb).astype(np.float32)
    sins[:, :32] *= -1.0

    bp = (cpb if g == 0 else np.zeros_like(cpb)).reshape(8, 128).T.copy()

    r = np.arange(128)
    maskb = np.where(r[None, :] >= r[:, None], 0.0, -240.0)

    return {
        "hT": np.ascontiguousarray(hidden[b].T).astype(bft),
        "wqkv": np.ascontiguousarray(wqkv).astype(bft),
        "bqkv": np.ascontiguousarray(bqkv).astype(bft),
        "cosb": cos,
        "sinb": sins,
        "wp": np.ascontiguousarray(cpw[cs, :]).astype(bft),
        "bp": np.ascontiguousarray(bp.astype(np.float32)),
        "maskb": maskb.astype(bft),
        "identb": np.eye(128, dtype=np.float32).astype(bft),
        "ident": np.eye(128, dtype=np.float32),
        "ones_row": np.ones((1, 128), bft),
    }


_NC_CACHE = {}


def run(inputs, trace=False, **spmd_kwargs):
    """Shard, execute on 8 cores, unshard. Returns (output, BassKernelResults)."""
    if "nc" not in _NC_CACHE:
        _NC_CACHE["nc"] = build_attention_nc(num_devices=8)
    nc = _NC_CACHE["nc"]
    in_maps = [make_core_inputs(inputs, c) for c in range(8)]
    res = run_bass_kernel_spmd(nc, in_maps, core_ids=list(range(8)),
                               trace=trace, **spmd_kwargs)
    outs = []
    for b in range(2):
        acc = np.zeros((D, S), np.float64)
        for g in range(4):
            acc += res.results[b * 4 + g]["outT"].astype(np.float64)
        outs.append(acc.T.astype(np.float32))
    return np.stack(outs, axis=0), res


def kernel(**inputs) -> np.ndarray:
    out, _ = run(inputs, trace=False)
    return out


# revision 23
# speedup vs baseline: 1.2112x; 1.1859x over previous
"""TRN2 Bass kernel for GPT-style causal self-attention with RoPE.

Reference (B=2, S=2048, D=1024, H=16, dk=64):
  qkv = hidden @ c_attn_w + c_attn_b; rope(q), rope(k) via position_ids;
  out = softmax(causal(q k^T / 8)) v, merged heads, @ c_proj_w + c_proj_b.

Sharding across 8 NeuronCores: core c = 4*b + g handles batch b and head
group g (4 heads = 256 dims). Each core computes its full S x S attention
and a row-sliced c_proj partial; the host sums the 4 partials per batch.

All matmul operands are bf16 (host converts inputs); PSUM stays f32.
Pipeline per core:
  1. QKV in natural layout (lhsT = host-transposed hidden chunks, DMAs
     issued in parallel from four engine queues); bias via K=1 ones-row
     matmul; rope via 2 DVE multiplies (pair-swap AP, trig broadcast over
     heads) + DVE add; one f32r PE transpose per 128-block; PSUM evictions
     cast to bf16 (Scalar).
  2. Attention blocks (c, hp) in DESCENDING size order, software-pipelined
     at kb granularity: PE alternates scores(i, kb) with PV(i-1, kb) so exp
     (Scalar) is never the serializer. Scores via K=64 matmul pairs (two
     heads in PE quadrants); causal mask folded in as a -240 bias PSUM
     accumulate matmul on the diagonal blocks; exp on Scalar (scale=1/8,
     bf16 out); PV accumulates [v | ones] so row 64 = softmax denominators;
     o_p evicted to SBUF immediately (frees the PSUM slot), then normalize
     via reciprocal_approx_fast + partition_broadcast + DVE multiply into
     per-chunk aT2 tiles (h2=1 via small DMA partition hop).
  3. Projection per 512-col chunk interleaved into the attention stream as
     soon as both head-pairs of that chunk are normalized; c_proj bias added
     by DVE tensor_scalar during PSUM eviction; bf16 output DMA per chunk.
Output per core: outT [1024, 2048] bf16 partial; host sums, transposes.
"""

from contextlib import ExitStack

import numpy as np
import ml_dtypes

import concourse.bacc as bacc
import concourse.tile as tile
import concourse.mybir as mybir
from concourse.bass_utils import run_bass_kernel_spmd

f32 = mybir.dt.float32
f32r = mybir.dt.float32r
bf16 = mybir.dt.bfloat16
AF = mybir.ActivationFunctionType
ALU = mybir.AluOpType

S = 2048
D = 1024
HD = 256           # head dims per core (4 heads x 64)
SB = S // 128      # 16
KC = D // 128      # 8
NCH = S // 512     # 4


def build_attention_nc(num_devices=8):
    nc = bacc.Bacc("TRN2", target_bir_lowering=False, debug=False,
                   num_devices=num_devices)

    hT_d = nc.dram_tensor("hT", [D, S], bf16, kind="ExternalInput")
    wqkv_d = nc.dram_tensor("wqkv", [D, 768], bf16, kind="ExternalInput")
    bqkv_d = nc.dram_tensor("bqkv", [1, 768], bf16, kind="ExternalInput")
    cos_d = nc.dram_tensor("cosb", [S, 64], f32, kind="ExternalInput")
    sin_d = nc.dram_tensor("sinb", [S, 64], f32, kind="ExternalInput")
    wp_d = nc.dram_tensor("wp", [HD, D], bf16, kind="ExternalInput")
    bp_d = nc.dram_tensor("bp", [128, 8], f32, kind="ExternalInput")
    maskb_d = nc.dram_tensor("maskb", [128, 128], bf16, kind="ExternalInput")
    identb_d = nc.dram_tensor("identb", [128, 128], bf16, kind="ExternalInput")
    ident_d = nc.dram_tensor("ident", [128, 128], f32r, kind="ExternalInput")
    onesrow_d = nc.dram_tensor("ones_row", [1, 128], bf16, kind="ExternalInput")
    outT_d = nc.dram_tensor("outT", [D, S], bf16, kind="ExternalOutput")

    with tile.TileContext(nc) as tc, ExitStack() as top:
        const = top.enter_context(tc.tile_pool(name="const", bufs=1))
        ident = const.tile([128, 128], f32r, tag="ident")
        maskb = const.tile([128, 128], bf16, tag="maskb")
        identb = const.tile([128, 128], bf16, tag="identb")
        ones_row = const.tile([1, 128], bf16, tag="ones_row")
        bp_sb = const.tile([128, 8], f32, tag="bp")

        persist = top.enter_context(tc.tile_pool(name="persist", bufs=1))
        qT = [persist.tile([128, S], bf16, tag=f"qT{hp}", name=f"qT{hp}")
              for hp in range(2)]
        kT = [persist.tile([128, S], bf16, tag=f"kT{hp}", name=f"kT{hp}")
              for hp in range(2)]
        v_sb = persist.tile([128, SB, 4, 65], bf16, tag="v")
        wp_sb = persist.tile([128, 2, D], bf16, tag="wp")
        # per-chunk attention outputs (transposed): rows 0:64 = head h2=0,
        # rows 64:128 = h2=1 (DMA hop from aTo)
        aT2 = [[persist.tile([128, 512], bf16, tag=f"aT2_{hp}_{c}",
                             name=f"aT2_{hp}_{c}") for c in range(NCH)]
               for hp in range(2)]
        aTo = [[persist.tile([64, 512], bf16, tag=f"aTo_{hp}_{c}",
                             name=f"aTo_{hp}_{c}") for c in range(NCH)]
               for hp in range(2)]

        # ============ stage 1: QKV + rope + transpose ============
        with ExitStack() as st1:
            hT_pool = st1.enter_context(tc.tile_pool(name="hT", bufs=1))
            w_pool = st1.enter_context(tc.tile_pool(name="w", bufs=1))
            trig_pool = st1.enter_context(tc.tile_pool(name="trig", bufs=4))
            rope_pool = st1.enter_context(tc.tile_pool(name="rope", bufs=1))
            qkv_ps = st1.enter_context(
                tc.tile_pool(name="qkv_ps", bufs=2, space="PSUM"))
            tr_ps = st1.enter_context(
                tc.tile_pool(name="tr_ps", bufs=2, space="PSUM"))

            # -------- input DMAs: one queue, strict priority order, with
            # the critical w/hT0 transfers split so compute starts at the
            # earliest chunk instead of after the full tensor
            w_sb = w_pool.tile([128, KC, 768], bf16, tag="w")
            hT_sb = [hT_pool.tile([128, KC, 512], bf16, tag=f"hT{sg}",
                                  name=f"hT{sg}") for sg in range(NCH)]
            trig = []
            for sg in range(NCH):
                cos_t = trig_pool.tile([128, 4, 64], f32, tag="cos",
                                       name=f"cos{sg}")
                sin_t = trig_pool.tile([128, 4, 64], f32, tag="sin",
                                       name=f"sin{sg}")
                trig.append((cos_t, sin_t))
            bqkv_sb = w_pool.tile([1, 768], bf16, tag="bqkv")
            nc.gpsimd.memset(v_sb[:, :, :, 64], 1.0)

            w_ap = wqkv_d.ap().rearrange("(kc p) f -> p kc f", kc=KC)

            def hT_ap(sg):
                return hT_d.ap()[:, sg * 512:(sg + 1) * 512].rearrange(
                    "(kc p) s -> p kc s", kc=KC)

            def trig_dma(sg):
                cos_t, sin_t = trig[sg]
                nc.sync.dma_start(
                    cos_t[:],
                    cos_d.ap()[sg * 512:(sg + 1) * 512, :].rearrange(
                        "(sbl p) d -> p sbl d", sbl=4))
                nc.sync.dma_start(
                    sin_t[:],
                    sin_d.ap()[sg * 512:(sg + 1) * 512, :].rearrange(
                        "(sbl p) d -> p sbl d", sbl=4))

            nc.sync.dma_start(w_sb[:, 0:4, :], w_ap[:, 0:4, :])
            nc.sync.dma_start(hT_sb[0][:, 0:4, :], hT_ap(0)[:, 0:4, :])
            trig_dma(0)
            nc.sync.dma_start(w_sb[:, 4:8, :], w_ap[:, 4:8, :])
            nc.sync.dma_start(hT_sb[0][:, 4:8, :], hT_ap(0)[:, 4:8, :])
            nc.sync.dma_start(bqkv_sb[:], bqkv_d.ap())
            nc.sync.dma_start(ones_row[:], onesrow_d.ap())
            nc.sync.dma_start(hT_sb[1][:], hT_ap(1))
            nc.sync.dma_start(ident[:], ident_d.ap())
            trig_dma(1)
            nc.sync.dma_start(hT_sb[2][:], hT_ap(2))
            trig_dma(2)
            nc.sync.dma_start(hT_sb[3][:], hT_ap(3))
            trig_dma(3)
            nc.sync.dma_start(maskb[:], maskb_d.ap())
            nc.sync.dma_start(identb[:], identb_d.ap())
            nc.sync.dma_start(
                wp_sb[:], wp_d.ap().rearrange("(kc2 p) f -> p kc2 f", kc2=2))
            nc.sync.dma_start(bp_sb[:], bp_d.ap())

            for sg in range(NCH):
                cos_t, sin_t = trig[sg]
                rope_tiles = {}
                for sbl in range(4):
                    sb = sg * 4 + sbl
                    qkv_p = qkv_ps.tile([128, 768], f32, tag="qkv_p")
                    for kc in range(KC):
                        lhsT = hT_sb[sg][:, kc, sbl * 128:(sbl + 1) * 128]
                        nc.tensor.matmul(qkv_p[:, 0:512], lhsT,
                                         w_sb[:, kc, 0:512],
                                         start=(kc == 0), stop=False)
                        nc.tensor.matmul(qkv_p[:, 512:768], lhsT,
                                         w_sb[:, kc, 512:768],
                                         start=(kc == 0), stop=False)
                    nc.tensor.matmul(qkv_p[:, 0:512], ones_row[:],
                                     bqkv_sb[:, 0:512], start=False, stop=True)
                    nc.tensor.matmul(qkv_p[:, 512:768], ones_row[:],
                                     bqkv_sb[:, 512:768], start=False,
                                     stop=True)

                    cos_b = cos_t[:, sbl:sbl + 1, :].broadcast_to((128, 4, 64))
                    sin_b = sin_t[:, sbl:sbl + 1, :].rearrange(
                        "p o (t d) -> p o t d", t=2).broadcast_to(
                        (128, 4, 2, 32))
                    for qk in range(2):
                        base = qk * HD
                        pin = qkv_p[:, base:base + HD]
                        pin_h = pin.rearrange("p (h d) -> p h d", h=4)
                        pin_sw = pin.rearrange("p (h t d) -> p h t d",
                                               h=4, t=2)[:, :, ::-1, :]
                        t1 = rope_pool.tile([128, HD], f32r,
                                            tag=f"t1_{qk}_{sbl}",
                                            name=f"t1_{qk}_{sbl}")
                        t2 = rope_pool.tile([128, HD], f32r,
                                            tag=f"t2_{qk}_{sbl}",
                                            name=f"t2_{qk}_{sbl}")
                        tt = rope_pool.tile([128, HD], f32r,
                                            tag=f"t_{qk}_{sbl}",
                                            name=f"t_{qk}_{sbl}")
                        nc.vector.tensor_tensor(
                            t1[:].rearrange("p (h d) -> p h d", h=4),
                            pin_h, cos_b, op=ALU.mult)
                        nc.vector.tensor_tensor(
                            t2[:].rearrange("p (h t d) -> p h t d", h=4, t=2),
                            pin_sw, sin_b, op=ALU.mult)
                        nc.gpsimd.tensor_tensor(tt[:], t1[:], t2[:],
                                                op=ALU.add)
                        rope_tiles[(qk, sbl)] = tt

                    nc.scalar.copy(
                        v_sb[:, sb, :, 0:64],
                        qkv_p[:, 512:768].rearrange("p (h d) -> p h d", h=4))

                for qk in range(2):
                    dest = qT if qk == 0 else kT
                    for hp in range(2):
                        tp = tr_ps.tile([128, 512], f32, tag="tr_p")
                        for sbl in range(4):
                            tt = rope_tiles[(qk, sbl)]
                            nc.tensor.matmul(
                                tp[:, sbl * 128:(sbl + 1) * 128].bitcast(f32r),
                                tt[:, hp * 128:(hp + 1) * 128], ident[:],
                                is_transpose=True, start=True, stop=True)
                        nc.scalar.copy(
                            dest[hp][:, sg * 512:(sg + 1) * 512], tp[:])

        # ============ stages 2+3 interleaved ============
        with ExitStack() as st2:
            pt_pool = st2.enter_context(tc.tile_pool(name="pt", bufs=33))
            nrm_pool = st2.enter_context(tc.tile_pool(name="nrm", bufs=4))
            po_pool = st2.enter_context(tc.tile_pool(name="po", bufs=2))
            st_ps = st2.enter_context(
                tc.tile_pool(name="st_ps", bufs=2, space="PSUM"))
            out_ps = st2.enter_context(
                tc.tile_pool(name="out_ps", bufs=2, space="PSUM"))
            pj_ps = st2.enter_context(
                tc.tile_pool(name="pj_ps", bufs=2, space="PSUM"))

            def emit_score(c, hp, kb, pts):
                q0 = max(512 * c, 128 * kb)
                off = q0 - 512 * c
                diag = 128 * kb >= 512 * c
                st_p = st_ps.tile([128, 2, 512], f32, tag="st_p")
                for h2 in range(2):
                    nc.tensor.matmul(
                        st_p[:, h2, off:512],
                        kT[hp][h2 * 64:(h2 + 1) * 64,
                               kb * 128:(kb + 1) * 128],
                        qT[hp][h2 * 64:(h2 + 1) * 64,
                               q0:512 * (c + 1)],
                        start=True, stop=not diag,
                        tile_position=(h2 * 64, 0))
                    if diag:
                        # causal mask: add -240 above the diagonal, pre-exp
                        nc.tensor.matmul(
                            st_p[:, h2, off:off + 128],
                            identb[:], maskb[:],
                            start=False, stop=True)
                pt = pt_pool.tile([128, 2, 512], bf16, tag="pt")
                nc.scalar.activation(pt[:, :, off:512],
                                     st_p[:, :, off:512],
                                     AF.Exp, scale=0.125)
                pts.append((kb, off, pt))

            def emit_pv(hp, kb, off, pt, o_ps, nkb):
                for h2 in range(2):
                    nc.tensor.matmul(
                        o_ps[h2][0:65, off:512],
                        v_sb[:, kb, 2 * hp + h2, :],
                        pt[:, h2, off:512],
                        start=(kb == 0), stop=(kb == nkb - 1))

            def emit_norm(c, hp, o_ps):
                # stage-parallel h2 chains, h2=1 leading (it has the extra
                # partition-hop DMA at the end)
                oes, den0s, rcp0s, bcs = {}, {}, {}, {}
                for h2 in (1, 0):
                    oe = nrm_pool.tile([65, 512], f32, tag="oe")
                    # free the PSUM slot fast, then normalize from SBUF
                    nc.vector.tensor_copy(oe[0:65, :], o_ps[h2][0:65, :])
                    oes[h2] = oe
                    den0s[h2] = nrm_pool.tile([1, 512], f32, tag="den0",
                                              name=f"den0_{h2}")
                    rcp0s[h2] = nrm_pool.tile([1, 512], f32, tag="rcp0",
                                              name=f"rcp0_{h2}")
                    bcs[h2] = nrm_pool.tile([64, 512], f32, tag="bc",
                                            name=f"bc_{h2}")
                for h2 in (1, 0):
                    # custom-DVE recip and partition_broadcast need
                    # partition-0 operands; DMA does the cross-partition hop
                    nc.gpsimd.dma_start(den0s[h2][:], oes[h2][64:65, :])
                for h2 in (1, 0):
                    nc.vector.reciprocal_approx_fast(rcp0s[h2][:],
                                                     den0s[h2][:])
                for h2 in (1, 0):
                    nc.gpsimd.partition_broadcast(bcs[h2][:], rcp0s[h2][:])
                for h2 in (1, 0):
                    out_ap = (aT2[hp][c][0:64, :] if h2 == 0
                              else aTo[hp][c][0:64, :])
                    nc.vector.tensor_tensor(out_ap, oes[h2][0:64, :],
                                            bcs[h2][:], op=ALU.mult)
                    if h2 == 1:
                        nc.gpsimd.dma_start(aT2[hp][c][64:128, :],
                                            aTo[hp][c][0:64, :])

            def emit_proj(c):
                po = po_pool.tile([128, 8, 512], bf16, tag="po")
                for dd in range(8):
                    pp = pj_ps.tile([128, 512], f32, tag="pp")
                    for kc2 in range(2):
                        nc.tensor.matmul(
                            pp[:],
                            wp_sb[:, kc2, dd * 128:(dd + 1) * 128],
                            aT2[kc2][c][:],
                            start=(kc2 == 0), stop=(kc2 == 1))
                    nc.vector.tensor_scalar_add(po[:, dd, :], pp[:],
                                                bp_sb[:, dd:dd + 1])
                    nc.sync.dma_start(
                        outT_d.ap()[dd * 128:(dd + 1) * 128,
                                    c * 512:(c + 1) * 512],
                        po[:, dd, :])

            # ascending block size: exp demand ramps up in step with the
            # PV backlog, so the st-buffer throttle never leaves PE idle;
            # scores(i) interleaved with PV(i-1) at kb granularity so
            # Scalar(exp) never stalls PE; ready proj chunks are dropped
            # into the middle of the kb stream to fill exp bubbles
            blocks = [(c, hp) for c in (0, 1, 2, 3) for hp in (0, 1)]
            prev = None
            proj_q = []
            for i in range(len(blocks) + 1):
                blk = blocks[i] if i < len(blocks) else None
                nkb_i = 4 * blk[0] + 4 if blk else 0
                nkb_p = len(prev[2]) if prev else 0
                o_ps = None
                if prev:
                    o_ps = [out_ps.tile([128, 512], f32, tag="o_p",
                                        name=f"o_p{h2}") for h2 in range(2)]
                pts = []
                proj_c = proj_q.pop(0) if proj_q else None
                for kb in range(max(nkb_i, nkb_p)):
                    if kb < nkb_i:
                        emit_score(blk[0], blk[1], kb, pts)
                    if kb < nkb_p:
                        pkb, poff, ppt = prev[2][kb]
                        emit_pv(prev[1], pkb, poff, ppt, o_ps, nkb_p)
                        if kb == nkb_p - 1:
                            # norm chain queued right behind the last PV so
                            # later GpSimd/DVE work can't head-of-line it
                            emit_norm(prev[0], prev[1], o_ps)
                    if kb == 5 and proj_c is not None:
                        emit_proj(proj_c)
                        proj_c = None
                if proj_c is not None:
                    emit_proj(proj_c)
                if prev:
                    if prev[1] == 1:
                        proj_q.append(prev[0])
                prev = (blk[0], blk[1], pts) if blk else None
            while proj_q:
                emit_proj(proj_q.pop(0))

    nc.finalize()
    return nc


def make_core_inputs(inputs, core):
    """Host-side shard prep for one core."""
    b, g = core // 4, core % 4
    hidden = np.asarray(inputs["hidden_states"], dtype=np.float32)
    pos = np.asarray(inputs["position_ids"])
    caw = np.asarray(inputs["c_attn_w"], dtype=np.float32)
    cab = np.asarray(inputs["c_attn_b"], dtype=np.float32)
    cpw = np.asarray(inputs["c_proj_w"], dtype=np.float32)
    cpb = np.asarray(inputs["c_proj_b"], dtype=np.float32)
    bft = ml_dtypes.bfloat16

    cs = slice(g * HD, (g + 1) * HD)
    wqkv = np.concatenate(
        [caw[:, cs], caw[:, D + g * HD:D + (g + 1) * HD],
         caw[:, 2 * D + g * HD:2 * D + (g + 1) * HD]], axis=1)
    bqkv = np.concatenate(
        [cab[cs], cab[D + g * HD:D + (g + 1) * HD],
         cab[2 * D + g * HD:2 * D + (g + 1) * HD]])[None, :]

    inv_freq = (1.0 / (10000.0 **
                       (np.arange(0, 64, 2, dtype=np.float64) / 64.0)))
    freqs = pos[b].astype(np.float64)[:, None] * inv_freq[None, :]
    emb = np.concatenate([freqs, freqs], axis=1)
    cos = np.cos(emb).astype(np.float32)
    sins = np.sin(emb).astype(np.float32)
    sins[:, :32] *= -1.0

    bp = (cpb if g == 0 else np.zeros_like(cpb)).reshape(8, 128).T.copy()

    r = np.arange(128)
    maskb = np.where(r[None, :] >= r[:, None], 0.0, -240.0)

    return {
        "hT": np.ascontiguousarray(hidden[b].T).astype(bft),
        "wqkv": np.ascontiguousarray(wqkv).astype(bft),
        "bqkv": np.ascontiguousarray(bqkv).astype(bft),
        "cosb": cos,
        "sinb": sins,
        "wp": np.ascontiguousarray(cpw[cs, :]).astype(bft),
        "bp": np.ascontiguousarray(bp.astype(np.float32)),
        "maskb": maskb.astype(bft),
        "identb": np.eye(128, dtype=np.float32).astype(bft),
        "ident": np.eye(128, dtype=np.float32),
        "ones_row": np.ones((1, 128), bft),
    }


_NC_CACHE = {}


def run(inputs, trace=False, **spmd_kwargs):
    """Shard, execute on 8 cores, unshard. Returns (output, BassKernelResults)."""
    if "nc" not in _NC_CACHE:
        _NC_CACHE["nc"] = build_attention_nc(num_devices=8)
    nc = _NC_CACHE["nc"]
    in_maps = [make_core_inputs(inputs, c) for c in range(8)]
    res = run_bass_kernel_spmd(nc, in_maps, core_ids=list(range(8)),
                               trace=trace, **spmd_kwargs)
    outs = []
    for b in range(2):
        acc = np.zeros((D, S), np.float64)
        for g in range(4):
            acc += res.results[b * 4 + g]["outT"].astype(np.float64)
        outs.append(acc.T.astype(np.float32))
    return np.stack(outs, axis=0), res


def kernel(**inputs) -> np.ndarray:
    out, _ = run(inputs, trace=False)
    return out
